# revision 8
# baseline (speedup 1.0000x reference)
"""Bass/Trainium2 kernel for nn_Attention_Layer (B=8, L=2048, D=1024).

Strategy (V1): pure data-parallel over batch — core c computes the full
attention layer for batch element c.

Per-core pipeline (everything on-chip after one load of x^T and weights):
  1. Projections on TensorE (fp16 in, fp32 PSUM accum):
       QT[e,l] = WqT.T @ xT   (lhsT = WqT[d,e] tile, rhs = xT[d,l])  -> fp16
       KT[e,l] likewise                                              -> fp16
       V[l,e]  = xT.T @ WvT   (lhsT = xT[d,l] tile, rhs = WvT[d,e])  -> bf16
  2. Scores transposed: ST[k,q] = KT.T @ QT (contract over e).
     Masking+softmax numerator fused into one ScalarE activation:
       E[k,q] = exp(ST[k,q] + bias[k]),  bias[k] = -44 (valid) / -1e30 (masked)
     The -44 shift keeps exp in comfortable fp32 range (scores reach ~±65);
     it cancels in U/r. E stored bf16 (full fp32 exponent range).
  3. U[q,e] = E.T @ V and r[q] = E.T @ ones (contract over k, on TensorE).
  4. out[q,e] = U[q,e] * (1/r[q]) on VectorE; DMA out fp32.

No per-row max subtraction is needed: scores are O(60) so exp stays finite
in fp32, and the reference's -2^31 padding value is reproduced exactly by
the additive -1e30 mask (exp -> 0).
"""

import os

import numpy as np

import concourse.bass as bass
import concourse.tile as tile
import concourse.bacc as bacc
from concourse import mybir
from concourse.bass_utils import run_bass_kernel_spmd

B, L, D = 8, 2048, 1024
P = 128
NDT = D // P   # 8 d-tiles (contraction tiles for projections)
NET = D // P   # 8 e-tiles (feature tiles)
NKT = L // P   # 16 k-tiles (key tiles)
NQT = L // P   # 16 q-tiles
QB = 512       # q-block width for the score matmuls
NQB = L // QB  # 4
MASK_SHIFT = -44.0
MASK_NEG = -1.0e30

f16 = mybir.dt.float16
bf16 = mybir.dt.bfloat16
f32 = mybir.dt.float32

LAST_RESULT = None
_NC_CACHE = {}


def _build_v1():
    nc = bacc.Bacc("TRN2", target_bir_lowering=False, debug=False, num_devices=B)

    xT_d = nc.dram_tensor("xT", [D, L], f16, kind="ExternalInput").ap()
    wqT_d = nc.dram_tensor("wqT", [D, D], f16, kind="ExternalInput").ap()
    wkT_d = nc.dram_tensor("wkT", [D, D], f16, kind="ExternalInput").ap()
    wvT_d = nc.dram_tensor("wvT", [D, D], f16, kind="ExternalInput").ap()
    maskT_d = nc.dram_tensor("maskT", [P, NKT], f32, kind="ExternalInput").ap()
    out_d = nc.dram_tensor("out", [L, D], f32, kind="ExternalOutput").ap()

    Exp = mybir.ActivationFunctionType.Exp

    with tile.TileContext(nc) as tc:
        with tc.tile_pool(name="qkv", bufs=1) as qkv_pool, \
             tc.tile_pool(name="cst", bufs=1) as cst_pool:
            # Long-lived tensors for the attention phase.
            QT = [qkv_pool.tile([P, L], f16, name=f"QT{i}", tag=f"QT{i}") for i in range(NET)]
            KT = [qkv_pool.tile([P, L], f16, name=f"KT{i}", tag=f"KT{i}") for i in range(NET)]
            V = [qkv_pool.tile([P, D], bf16, name=f"V{i}", tag=f"V{i}") for i in range(NKT)]
            maskT = cst_pool.tile([P, NKT], f32, name="maskT", tag="maskT")
            ones = cst_pool.tile([P, 1], bf16, name="ones", tag="ones")
            nc.sync.dma_start(maskT[:], maskT_d[:, :])
            nc.vector.memset(ones[:], 1.0)

            # ---- Phase 1: projections ----
            with tc.tile_pool(name="xw", bufs=1) as xw_pool, \
                 tc.tile_pool(name="pproj", bufs=4, space="PSUM") as pproj:
                xT = [xw_pool.tile([P, L], f16, name=f"xT{i}", tag=f"xT{i}") for i in range(NDT)]
                wq = [xw_pool.tile([P, D], f16, name=f"wq{i}", tag=f"wq{i}") for i in range(NDT)]
                wk = [xw_pool.tile([P, D], f16, name=f"wk{i}", tag=f"wk{i}") for i in range(NDT)]
                wv = [xw_pool.tile([P, D], f16, name=f"wv{i}", tag=f"wv{i}") for i in range(NDT)]
                for i in range(NDT):
                    sl = slice(i * P, (i + 1) * P)
                    nc.sync.dma_start(xT[i][:], xT_d[sl, :])
                    nc.sync.dma_start(wq[i][:], wqT_d[sl, :])
                    nc.sync.dma_start(wk[i][:], wkT_d[sl, :])
                    nc.sync.dma_start(wv[i][:], wvT_d[sl, :])

                # QT / KT: out[e-tile, l-block]
                for w_t, dstT in ((wq, QT), (wk, KT)):
                    for et in range(NET):
                        for lb in range(L // QB):
                            ps = pproj.tile([P, QB], f32, name="pp", tag="pp")
                            for dt_ in range(NDT):
                                nc.tensor.matmul(
                                    ps[:],
                                    lhsT=w_t[dt_][:, et * P:(et + 1) * P],
                                    rhs=xT[dt_][:, lb * QB:(lb + 1) * QB],
                                    start=(dt_ == 0), stop=(dt_ == NDT - 1),
                                )
                            nc.vector.tensor_copy(
                                dstT[et][:, lb * QB:(lb + 1) * QB], ps[:])
                # V: out[l-tile, e-block]
                for lt in range(NQT):
                    for eb in range(D // QB):
                        ps = pproj.tile([P, QB], f32, name="pp", tag="pp")
                        for dt_ in range(NDT):
                            nc.tensor.matmul(
                                ps[:],
                                lhsT=xT[dt_][:, lt * P:(lt + 1) * P],
                                rhs=wv[dt_][:, eb * QB:(eb + 1) * QB],
                                start=(dt_ == 0), stop=(dt_ == NDT - 1),
                            )
                        nc.vector.tensor_copy(
                            V[lt][:, eb * QB:(eb + 1) * QB], ps[:])

            # ---- Phase 2: attention ----
            with tc.tile_pool(name="attn", bufs=2) as attn_pool, \
                 tc.tile_pool(name="outp", bufs=3) as outp, \
                 tc.tile_pool(name="small", bufs=4) as small, \
                 tc.tile_pool(name="ps_s", bufs=2, space="PSUM") as ps_s, \
                 tc.tile_pool(name="ps_u", bufs=2, space="PSUM") as ps_u, \
                 tc.tile_pool(name="ps_r", bufs=1, space="PSUM") as ps_r:
                for qb in range(NQB):
                    qsl = slice(qb * QB, (qb + 1) * QB)
                    E = attn_pool.tile([P, NKT, QB], bf16, name="E", tag="E")
                    for kt in range(NKT):
                        ps = ps_s.tile([P, QB], f32, name="ps", tag="ps")
                        for et in range(NET):
                            nc.tensor.matmul(
                                ps[:],
                                lhsT=KT[et][:, kt * P:(kt + 1) * P],
                                rhs=QT[et][:, qsl],
                                start=(et == 0), stop=(et == NET - 1),
                            )
                        nc.scalar.activation(
                            E[:, kt, :], ps[:], Exp,
                            bias=maskT[:, kt:kt + 1], scale=1.0)
                    for qt in range(QB // P):
                        q0 = qb * QB + qt * P  # global q row start
                        psU = ps_u.tile([P, D], f32, name="psU", tag="psU")
                        psr = ps_r.tile([P, 1], f32, name="psr", tag="psr")
                        for kt in range(NKT):
                            lhsT = E[:, kt, qt * P:(qt + 1) * P]
                            st, sp = (kt == 0), (kt == NKT - 1)
                            nc.tensor.matmul(psU[:, 0:QB], lhsT=lhsT,
                                             rhs=V[kt][:, 0:QB],
                                             start=st, stop=sp)
                            nc.tensor.matmul(psU[:, QB:D], lhsT=lhsT,
                                             rhs=V[kt][:, QB:D],
                                             start=st, stop=sp)
                            nc.tensor.matmul(psr[:], lhsT=lhsT, rhs=ones[:],
                                             start=st, stop=sp)
                        rinv = small.tile([P, 1], f32, name="rinv", tag="rinv")
                        nc.vector.reciprocal(rinv[:], psr[:])
                        ob = outp.tile([P, D], f32, name="ob", tag="ob")
                        nc.vector.tensor_scalar_mul(ob[:, 0:QB], psU[:, 0:QB], rinv[:])
                        nc.vector.tensor_scalar_mul(ob[:, QB:D], psU[:, QB:D], rinv[:])
                        nc.sync.dma_start(out_d[q0:q0 + P, :], ob[:])

    nc.compile()
    return nc


def _build_v2(nk):
    """Balanced variant. nk[b] = ceil(lens[b]/128) k-tiles per batch.

    - KV projection split into (batch, k-tile) units, spread uniformly over
      cores (UPC units each, padded with dummies); results all-gathered.
    - Every core computes Q projection + attention for one 256-row q-chunk
      of EVERY batch (core c takes rows [256c, 256c+256) of each batch), so
      per-core attention work is identical by construction; masked k-tiles
      (beyond nk[b]) are skipped statically.
    """
    QW = L // B  # 256 q rows per (core, batch) slot
    units = [(b, kt) for b in range(B) for kt in range(nk[b])]
    n_real = len(units)
    UPC = (n_real + B - 1) // B
    units = units + [units[0]] * (B * UPC - n_real)
    slot_of = {}
    for i, u in enumerate(units[:n_real]):
        slot_of[u] = (i // UPC, i % UPC)

    nc = bacc.Bacc("TRN2", target_bir_lowering=False, debug=False, num_devices=B)

    xkv_d = nc.dram_tensor("xkv", [D, UPC * P], f16, kind="ExternalInput").ap()
    xqT_d = nc.dram_tensor("xqT", [D, L], f16, kind="ExternalInput").ap()
    wqT_d = nc.dram_tensor("wqT", [D, D], f16, kind="ExternalInput").ap()
    wkT_d = nc.dram_tensor("wkT", [D, D], f16, kind="ExternalInput").ap()
    wvT_d = nc.dram_tensor("wvT", [D, D], f16, kind="ExternalInput").ap()
    maskT_d = nc.dram_tensor("maskT", [P, B * NKT], f32, kind="ExternalInput").ap()
    out_d = nc.dram_tensor("out", [L, D], f32, kind="ExternalOutput").ap()

    Exp = mybir.ActivationFunctionType.Exp
    UG = (UPC + 3) // 4  # psum unit-groups of 4 for the KT-piece matmuls

    with tile.TileContext(nc) as tc:
        with tc.tile_pool(name="res", bufs=1) as res_pool, \
             tc.tile_pool(name="dram", bufs=1, space="DRAM") as dram_pool:
            QT = [res_pool.tile([P, L], f16, name=f"QT{i}", tag=f"QT{i}")
                  for i in range(NET)]
            maskT = res_pool.tile([P, B * NKT], f32, name="maskT", tag="maskT")
            ones = res_pool.tile([P, 1], bf16, name="ones", tag="ones")
            nc.sync.dma_start(maskT[:], maskT_d[:, :])
            nc.vector.memset(ones[:], 1.0)

            kt_src = dram_pool.tile([UPC, P, NET, P], f16, name="kt_src")
            v_src = dram_pool.tile([UPC, P, D], bf16, name="v_src")
            kt_all = dram_pool.tile([B * UPC, P, NET, P], f16, name="kt_all",
                                    addr_space="Shared")
            v_all = dram_pool.tile([B * UPC, P, D], bf16, name="v_all",
                                   addr_space="Shared")

            # ---- Phase A: KV projection units + Phase C: Q projection ----
            with tc.tile_pool(name="xw", bufs=1) as xw_pool, \
                 tc.tile_pool(name="kvs", bufs=3) as kvs_pool, \
                 tc.tile_pool(name="ppk", bufs=4, space="PSUM") as ppk_pool, \
                 tc.tile_pool(name="pp", bufs=3, space="PSUM") as pp:
                xkv = [xw_pool.tile([P, UPC * P], f16, name=f"xkv{i}",
                                    tag=f"xkv{i}") for i in range(NDT)]
                wk = [xw_pool.tile([P, D], f16, name=f"wk{i}", tag=f"wk{i}")
                      for i in range(NDT)]
                wv = [xw_pool.tile([P, D], f16, name=f"wv{i}", tag=f"wv{i}")
                      for i in range(NDT)]
                wq = [xw_pool.tile([P, D], f16, name=f"wq{i}", tag=f"wq{i}")
                      for i in range(NDT)]
                xqT = [xw_pool.tile([P, L], f16, name=f"xqT{i}", tag=f"xqT{i}")
                       for i in range(NDT)]
                for i in range(NDT):
                    sl = slice(i * P, (i + 1) * P)
                    nc.sync.dma_start(xkv[i][:], xkv_d[sl, :])
                    nc.sync.dma_start(wk[i][:], wkT_d[sl, :])
                    nc.sync.dma_start(wv[i][:], wvT_d[sl, :])
                    nc.sync.dma_start(wq[i][:], wqT_d[sl, :])
                    nc.sync.dma_start(xqT[i][:], xqT_d[sl, :])

                # KT pieces. One PSUM bank per unit accumulation group —
                # interleaved groups in one bank break on start=True's
                # bank clear.
                ktp = [kvs_pool.tile([P, NET, 4 * P], f16, name=f"ktp{g}",
                                     tag=f"ktp{g}", bufs=1)
                       for g in range(UG)]
                for et in range(NET):
                    for j in range(UPC):
                        psu = ppk_pool.tile([P, P], f32, name="ppk", tag="ppk")
                        for dt_ in range(NDT):
                            nc.tensor.matmul(
                                psu[:],
                                lhsT=wk[dt_][:, et * P:(et + 1) * P],
                                rhs=xkv[dt_][:, j * P:(j + 1) * P],
                                start=(dt_ == 0), stop=(dt_ == NDT - 1),
                            )
                        nc.vector.tensor_copy(
                            ktp[j // 4][:, et, (j % 4) * P:(j % 4 + 1) * P],
                            psu[:])
                for j in range(UPC):
                    nc.sync.dma_start(
                        kt_src[j],
                        ktp[j // 4][:, :, (j % 4) * P:(j % 4 + 1) * P])

                # V pieces (N=512 matmuls, stationary = x k-slice).
                for j in range(UPC):
                    vp = kvs_pool.tile([P, D], bf16, name="vp", tag="vp", bufs=3)
                    for eb in range(2):
                        ps = pp.tile([P, QB], f32, name="ppv", tag="ppv")
                        for dt_ in range(NDT):
                            nc.tensor.matmul(
                                ps[:],
                                lhsT=xkv[dt_][:, j * P:(j + 1) * P],
                                rhs=wv[dt_][:, eb * QB:(eb + 1) * QB],
                                start=(dt_ == 0), stop=(dt_ == NDT - 1),
                            )
                        nc.vector.tensor_copy(vp[:, eb * QB:(eb + 1) * QB], ps[:])
                    nc.sync.dma_start(v_src[j], vp[:])

                # ---- Phase B: all-gather of KV pieces ----
                nc.gpsimd.collective_compute(
                    "AllGather", mybir.AluOpType.bypass,
                    replica_groups=[list(range(B))],
                    ins=[kt_src.opt()], outs=[kt_all.opt()])
                nc.gpsimd.collective_compute(
                    "AllGather", mybir.AluOpType.bypass,
                    replica_groups=[list(range(B))],
                    ins=[v_src.opt()], outs=[v_all.opt()])

                # ---- Phase C: Q projection ----
                for et in range(NET):
                    for lb in range(L // QB):
                        ps = pp.tile([P, QB], f32, name="ppq", tag="ppv")
                        for dt_ in range(NDT):
                            nc.tensor.matmul(
                                ps[:],
                                lhsT=wq[dt_][:, et * P:(et + 1) * P],
                                rhs=xqT[dt_][:, lb * QB:(lb + 1) * QB],
                                start=(dt_ == 0), stop=(dt_ == NDT - 1),
                            )
                        nc.vector.tensor_copy(
                            QT[et][:, lb * QB:(lb + 1) * QB], ps[:])

            # ---- Phase D: attention slots ----
            with tc.tile_pool(name="kio", bufs=4) as kio, \
                 tc.tile_pool(name="vio", bufs=4) as vio, \
                 tc.tile_pool(name="epool", bufs=4) as epool, \
                 tc.tile_pool(name="outp", bufs=3) as outp, \
                 tc.tile_pool(name="small", bufs=4) as small, \
                 tc.tile_pool(name="ps_s", bufs=2, space="PSUM") as ps_s, \
                 tc.tile_pool(name="ps_u", bufs=1, space="PSUM") as ps_u, \
                 tc.tile_pool(name="ps_r", bufs=1, space="PSUM") as ps_r:
                for b in range(B):
                    qsl = slice(b * QW, (b + 1) * QW)
                    psU = [ps_u.tile([P, D], f32, name=f"psU{qt}", tag=f"psU{qt}")
                           for qt in range(2)]
                    psr = [ps_r.tile([P, 1], f32, name=f"psr{qt}", tag=f"psr{qt}")
                           for qt in range(2)]
                    for kt in range(nk[b]):
                        cu, ju = slot_of[(b, kt)]
                        idx = cu * UPC + ju
                        ktbuf = kio.tile([P, NET, P], f16, name="ktbuf", tag="ktbuf")
                        vbuf = vio.tile([P, D], bf16, name="vbuf", tag="vbuf")
                        nc.sync.dma_start(ktbuf[:], kt_all[idx])
                        nc.sync.dma_start(vbuf[:], v_all[idx])
                        psS = ps_s.tile([P, QW], f32, name="psS", tag="psS")
                        for et in range(NET):
                            nc.tensor.matmul(
                                psS[:],
                                lhsT=ktbuf[:, et, :],
                                rhs=QT[et][:, qsl],
                                start=(et == 0), stop=(et == NET - 1),
                            )
                        E = epool.tile([P, QW], bf16, name="E", tag="E")
                        nc.scalar.activation(
                            E[:], psS[:], Exp,
                            bias=maskT[:, b * NKT + kt:b * NKT + kt + 1],
                            scale=1.0)
                        st, sp = (kt == 0), (kt == nk[b] - 1)
                        for qt in range(2):
                            lhsT = E[:, qt * P:(qt + 1) * P]
                            nc.tensor.matmul(psU[qt][:, 0:QB], lhsT=lhsT,
                                             rhs=vbuf[:, 0:QB],
                                             start=st, stop=sp)
                            nc.tensor.matmul(psU[qt][:, QB:D], lhsT=lhsT,
                                             rhs=vbuf[:, QB:D],
                                             start=st, stop=sp)
                            nc.tensor.matmul(psr[qt][:], lhsT=lhsT,
                                             rhs=ones[:],
                                             start=st, stop=sp)
                    for qt in range(2):
                        rinv = small.tile([P, 1], f32, name="rinv", tag="rinv")
                        nc.vector.reciprocal(rinv[:], psr[qt][:])
                        ob = outp.tile([P, D], f32, name="ob", tag="ob")
                        nc.vector.tensor_scalar_mul(ob[:, 0:QB],
                                                    psU[qt][:, 0:QB], rinv[:])
                        nc.vector.tensor_scalar_mul(ob[:, QB:D],
                                                    psU[qt][:, QB:D], rinv[:])
                        q0 = b * QW + qt * P
                        nc.sync.dma_start(out_d[q0:q0 + P, :], ob[:])

    nc.compile()
    return nc, units, UPC


def _get_nc():
    if "v1" not in _NC_CACHE:
        _NC_CACHE["v1"] = _build_v1()
    return _NC_CACHE["v1"]


def _kernel_v1(inputs, wqT, wkT, wvT, lens):
    global LAST_RESULT
    ar = np.arange(L, dtype=np.int64)
    in_maps = []
    for c in range(B):
        xT = np.ascontiguousarray(inputs[c].T).astype(np.float16)
        mask = np.where(ar < int(lens[c]), MASK_SHIFT, MASK_NEG).astype(np.float32)
        maskT = np.ascontiguousarray(mask.reshape(NKT, P).T)  # [P, NKT]
        in_maps.append({
            "xT": xT, "wqT": wqT, "wkT": wkT, "wvT": wvT, "maskT": maskT,
        })

    if "v1" not in _NC_CACHE:
        _NC_CACHE["v1"] = _build_v1()
    nc = _NC_CACHE["v1"]
    res = run_bass_kernel_spmd(nc, in_maps, core_ids=list(range(B)))
    LAST_RESULT = res
    out = np.stack([res.results[c]["out"] for c in range(B)], axis=0)
    return out.astype(np.float32)


def _kernel_v2(inputs, wqT, wkT, wvT, lens):
    global LAST_RESULT
    QW = L // B
    nk = tuple(max(1, min(NKT, -(-int(lens[b]) // P))) for b in range(B))
    key = ("v2", nk)
    if key not in _NC_CACHE:
        _NC_CACHE[key] = _build_v2(list(nk))
    nc, units, UPC = _NC_CACHE[key]

    xT = np.ascontiguousarray(inputs.transpose(0, 2, 1)).astype(np.float16)

    # mask bias table [P, B*NKT]: column b*NKT+kt = bias for batch b, k-tile kt
    ar = np.arange(L, dtype=np.int64)
    maskT = np.empty((P, B * NKT), dtype=np.float32)
    for b in range(B):
        m = np.where(ar < int(lens[b]), MASK_SHIFT, MASK_NEG).astype(np.float32)
        maskT[:, b * NKT:(b + 1) * NKT] = m.reshape(NKT, P).T

    in_maps = []
    for c in range(B):
        # KV-unit x slices for this core
        xkv = np.empty((D, UPC * P), dtype=np.float16)
        for j in range(UPC):
            b, kt = units[c * UPC + j]
            xkv[:, j * P:(j + 1) * P] = xT[b][:, kt * P:(kt + 1) * P]
        # q-chunk rows [QW*c, QW*(c+1)) of every batch, batch-major columns
        xqT = np.empty((D, L), dtype=np.float16)
        for b in range(B):
            xqT[:, b * QW:(b + 1) * QW] = xT[b][:, c * QW:(c + 1) * QW]
        in_maps.append({
            "xkv": xkv, "xqT": xqT,
            "wqT": wqT, "wkT": wkT, "wvT": wvT, "maskT": maskT,
        })

    res = run_bass_kernel_spmd(nc, in_maps, core_ids=list(range(B)))
    LAST_RESULT = res
    out = np.empty((B, L, D), dtype=np.float32)
    for c in range(B):
        oc = res.results[c]["out"]
        for b in range(B):
            out[b, c * QW:(c + 1) * QW, :] = oc[b * QW:(b + 1) * QW, :]
    return out


def kernel(inputs, Wq, Wk, Wv, lens):
    inputs = np.asarray(inputs, dtype=np.float32)
    Wq = np.asarray(Wq, dtype=np.float32)
    Wk = np.asarray(Wk, dtype=np.float32)
    Wv = np.asarray(Wv, dtype=np.float32)
    lens = np.asarray(lens, dtype=np.int32)

    wqT = np.ascontiguousarray(Wq.T).astype(np.float16)
    wkT = np.ascontiguousarray(Wk.T).astype(np.float16)
    wvT = np.ascontiguousarray(Wv.T).astype(np.float16)

    mode = os.environ.get("KERNEL_MODE", "v2")
    if mode == "v1":
        return _kernel_v1(inputs, wqT, wkT, wvT, lens)
    return _kernel_v2(inputs, wqT, wkT, wvT, lens)
